# revision 28
# baseline (speedup 1.0000x reference)
"""Causal attention + output projection on 8 Trainium2 NeuronCores.

Problem (hardcoded): B=2, H=12, T=2048, D=64, DIM=768, fp32.

Sharding: 24 (b, h) pairs -> 3 heads per core; cores 0-3 take b=0,
cores 4-7 take b=1.  Each core computes attention for its 3 heads plus
the partial output projection  sum_h y_h @ W[h*64:(h+1)*64, :]  as a
(T, DIM) partial; the host sums the 4 partials per batch.  No
collectives.

Device-side layout is fully transposed ([s, q]) so no on-chip
transposes are needed:
  - host feeds qT = q^T / sqrt(D) and kT = k^T packed in one tensor
  - host feeds biasT = bias^T with the causal mask pre-added
    (-1e4 on s > q) in bf16 (halves the dominant HBM traffic)
  - v is fed augmented with 64 ones-columns so a single PV matmul
    yields both y^T (rows 0:64) and the softmax denominators
    replicated across rows 64:128.

Per (head, q-chunk of 512, group of 4 s-tiles):
  PSUM[s=128, q=2048] <- identity-matmul copy of biasT (bf16)
  PSUM                += kT-tile.T @ qT-chunk   (fp32, causally trimmed)
  SBUF P = exp(PSUM)                            (one ACT instruction)
  PSUM_y[128, 512]    += vaug-tile.T @ P-slice  (accumulated over s)
then  rec = 1/sums  (DVE, partition-realigning read 64:128 -> 0:64),
      yT[:, chunk] = y_un * rec.
Projection: out[t-block, :] accumulates yT_h-slice.T @ W_h over heads.

This walrus build allows at most ONE DMA-lane semaphore wait per
hardware instruction, and only as the instruction's sole wait.  The
head loop is therefore a hardware `For_i` whose back-edge barrier
resets all semaphores: every SBUF slot is written exactly once per
iteration (10 distinct bias tiles per head), so no refill DMA ever
carries a slot-release wait, and tiny "absorb" matmuls keep each
DMA-completion wait alone on a throwaway instruction.
"""

import math

import numpy as np
import ml_dtypes

B, H, T, D = 2, 12, 2048, 64
DIM = H * D
NCORES = 8
HPC = 3           # heads per core
P = 128
QC = 512          # q-chunk width (one PSUM bank of fp32)
NJ = T // QC      # 4 q-chunks
NT = T // P       # 16 s-tiles
GROUP = 4         # s-tiles per PSUM logits group (4 banks)

_PROGRAM = None


def _build_program():
    import concourse.bass as bass
    import concourse.mybir as mybir
    import concourse.tile as tile
    from concourse import bacc
    from contextlib import ExitStack

    dt = mybir.dt
    f32 = dt.float32
    bf16 = dt.bfloat16
    EXP = mybir.ActivationFunctionType.Exp
    ds = bass.ds

    nc = bacc.Bacc("TRN2", num_devices=NCORES)
    # flat layouts so per-head slices are register-offset APs
    # per-head fused [va | qT(pad) | kT(pad)] block: one DMA per head
    comb = nc.declare_dram_parameter("comb", [HPC * P, 3 * T], f32, isOutput=False)
    biasT = nc.declare_dram_parameter("biasT", [HPC * 10 * GROUP * P, QC], bf16, isOutput=False)
    wproj = nc.declare_dram_parameter("wproj", [D, HPC * DIM], f32, isOutput=False)
    out = nc.declare_dram_parameter("out", [T, DIM], f32, isOutput=True)

    with tile.TileContext(nc) as tc, ExitStack() as ctx:
        from concourse.masks import make_identity

        const_pool = ctx.enter_context(tc.tile_pool(name="const", bufs=1))
        id_t = const_pool.tile([P, P], bf16)
        make_identity(nc, id_t[:])  # gpsimd memset+affine_select: no DMA lane

        w_pool = ctx.enter_context(tc.tile_pool(name="w", bufs=1))
        w_all = w_pool.tile([D, HPC * DIM], f32)
        nc.sync.dma_start(w_all[:], wproj[:])

        yT_pool = ctx.enter_context(tc.tile_pool(name="yT", bufs=1))
        yT_t = yT_pool.tile([D, HPC * T], f32)

        with (
            tc.tile_pool(name="head", bufs=1) as head_pool,
            tc.tile_pool(name="bias", bufs=1) as bias_pool,
            tc.tile_pool(name="pexp", bufs=2) as pexp_pool,
            tc.tile_pool(name="rec", bufs=2) as rec_pool,
            tc.tile_pool(name="psl", bufs=1, space="PSUM") as psl_pool,
            tc.tile_pool(name="psy", bufs=2, space="PSUM") as psy_pool,
        ):
            with tc.For_i(0, HPC, 1) as hreg:
                cb_t = head_pool.tile([P, 3 * T], f32)
                nc.sync.dma_start(cb_t[:], comb[ds(hreg * P, P), :])
                va_t = cb_t[:, 0:T]
                qT_t = cb_t[0:D, T : 2 * T]
                kT_t = cb_t[0:D, 2 * T : 3 * T]
                NREG = 10
                b_all = bias_pool.tile([P, NREG * GROUP * QC], bf16)
                nc.scalar.dma_start(
                    b_all[:, 0 : 3 * GROUP * QC].rearrange(
                        "p (a q) -> p a q", a=3 * GROUP
                    ),
                    biasT[
                        ds(hreg * (NREG * GROUP * P), 3 * GROUP * P), :
                    ].rearrange("(a p) q -> p a q", p=P),
                )
                nc.scalar.dma_start(
                    b_all[:, 3 * GROUP * QC :].rearrange(
                        "p (a q) -> p a q", a=7 * GROUP
                    ),
                    biasT[
                        ds(hreg * (NREG * GROUP * P) + 3 * GROUP * P,
                           7 * GROUP * P),
                        :,
                    ].rearrange("(a p) q -> p a q", p=P),
                )
                for j in range(NJ):
                    psy_t = psy_pool.tile([P, QC], f32)
                    for g in range(j + 1):
                        r = j * (j + 1) // 2 + g
                        b_t = b_all[:, r * GROUP * QC : (r + 1) * GROUP * QC]
                        psl_t = psl_pool.tile([P, GROUP * QC], f32)
                        for t in range(GROUP):
                            i = g * GROUP + t
                            # bias lands first (identity copy, clears bank)
                            nc.tensor.matmul(
                                psl_t[:, t * QC : (t + 1) * QC],
                                lhsT=id_t[:],
                                rhs=b_t[:, t * QC : (t + 1) * QC],
                                start=True,
                                stop=False,
                            )
                            # causally-trimmed QK accumulate on top
                            c0 = max(0, P * i - QC * j)
                            nc.tensor.matmul(
                                psl_t[:, t * QC + c0 : (t + 1) * QC],
                                lhsT=kT_t[:, i * P : (i + 1) * P],
                                rhs=qT_t[:, j * QC + c0 : (j + 1) * QC],
                                start=False,
                                stop=True,
                            )
                        pe_t = pexp_pool.tile([P, GROUP * QC], f32)
                        nc.scalar.activation(pe_t[:], psl_t[:], EXP)
                        for t in range(GROUP):
                            i = g * GROUP + t
                            nc.tensor.matmul(
                                psy_t[:],
                                lhsT=va_t[:, i * P : (i + 1) * P],
                                rhs=pe_t[:, t * QC : (t + 1) * QC],
                                start=(i == 0),
                                stop=(i == 4 * j + 3),
                            )
                    # rows 64:128 of psy hold the softmax denominators
                    # (replicated); realign to partitions 0:64 via the DVE
                    # output crossbar while taking the reciprocal.
                    rec_t = rec_pool.tile([D, QC], f32)
                    nc.vector.reciprocal(rec_t[:], psy_t[D : 2 * D, :])
                    nc.vector.tensor_mul(
                        yT_t[:, ds(hreg * T + j * QC, QC)],
                        psy_t[0:D, :],
                        rec_t[:],
                    )

        with (
            tc.tile_pool(name="psp", bufs=2, space="PSUM") as psp_pool,
            tc.tile_pool(name="outp", bufs=1) as out_pool,
        ):
            o_big = out_pool.tile([P, NT * DIM], f32)
            for tb in range(NT):
                psp_t = psp_pool.tile([P, DIM], f32)
                for o0, ow in ((0, 512), (512, 256)):
                    for h in range(HPC):
                        nc.tensor.matmul(
                            psp_t[:, o0 : o0 + ow],
                            lhsT=yT_t[:, h * T + tb * P : h * T + (tb + 1) * P],
                            rhs=w_all[:, h * DIM + o0 : h * DIM + o0 + ow],
                            start=(h == 0),
                            stop=(h == HPC - 1),
                        )
                nc.vector.tensor_copy(
                    o_big[:, tb * DIM : (tb + 1) * DIM], psp_t[:]
                )
                if tb == NT // 2 - 1:
                    nc.sync.dma_start(
                        out[0 : T // 2, :].rearrange("(a p) o -> p a o", p=P),
                        o_big[:, 0 : (NT // 2) * DIM].rearrange(
                            "p (a o) -> p a o", a=NT // 2
                        ),
                    )
            nc.sync.dma_start(
                out[T // 2 : T, :].rearrange("(a p) o -> p a o", p=P),
                o_big[:, (NT // 2) * DIM :].rearrange(
                    "p (a o) -> p a o", a=NT // 2
                ),
            )

    nc.finalize()
    return nc


def _get_program():
    global _PROGRAM
    if _PROGRAM is None:
        _PROGRAM = _build_program()
    return _PROGRAM


def make_in_maps(q, k, v, attn_bias, W_proj):
    """Host-side sharding/layout prep: one input map per core."""
    q = np.asarray(q, dtype=np.float32)
    k = np.asarray(k, dtype=np.float32)
    v = np.asarray(v, dtype=np.float32)
    attn_bias = np.asarray(attn_bias, dtype=np.float32)
    W_proj = np.asarray(W_proj, dtype=np.float32)

    scale = 1.0 / math.sqrt(D)
    # causal mask in transposed [s, q] coords: masked where s > q
    smask = (np.arange(T)[:, None] > np.arange(T)[None, :]).astype(np.float32)
    smask *= -10000.0
    w_heads = W_proj.reshape(H, D, DIM)

    in_maps = []
    for c in range(NCORES):
        b = c // 4
        h0 = HPC * (c % 4)
        hs = slice(h0, h0 + HPC)
        cb = np.zeros((HPC, P, 3 * T), dtype=np.float32)
        # va blocks: cb[:, :, k*128:(k+1)*128] = [v-tile | ones]
        va = cb[:, :, 0:T].reshape(HPC, P, NT, P)
        va[:, :, :, :D] = v[b, hs].reshape(HPC, NT, P, D).transpose(0, 2, 1, 3)
        va[:, :, :, D:] = 1.0
        cb[:, :D, T : 2 * T] = q[b, hs].transpose(0, 2, 1) * scale
        cb[:, :D, 2 * T : 3 * T] = k[b, hs].transpose(0, 2, 1)
        biasT = attn_bias[b, hs].transpose(0, 2, 1) + smask[None]
        biasT = biasT.astype(ml_dtypes.bfloat16)
        # pack the 10 causally-needed (j-chunk, s-group) regions of each
        # head contiguously: region (j, g) = rows [g*512:(g+1)*512] of
        # column chunk j
        regions = []
        for j in range(NJ):
            for g in range(j + 1):
                regions.append(
                    biasT[:, g * GROUP * P : (g + 1) * GROUP * P,
                          j * QC : (j + 1) * QC]
                )
        biasT = np.ascontiguousarray(
            np.concatenate(regions, axis=1)
        )
        in_maps.append(
            {
                "comb": cb.reshape(HPC * P, 3 * T),
                "biasT": biasT.reshape(HPC * 10 * GROUP * P, QC),
                "wproj": np.ascontiguousarray(
                    w_heads[hs].transpose(1, 0, 2).reshape(D, HPC * DIM)
                ),
            }
        )
    return in_maps


def assemble_output(results):
    """Sum the 4 per-core partial projections for each batch."""
    out = np.zeros((B, T, DIM), dtype=np.float32)
    for c in range(NCORES):
        out[c // 4] += results[c]["out"]
    return out


def kernel(q, k, v, attn_bias, W_proj):
    from concourse.bass_utils import run_bass_kernel_spmd

    nc = _get_program()
    in_maps = make_in_maps(q, k, v, attn_bias, W_proj)
    res = run_bass_kernel_spmd(nc, in_maps, list(range(NCORES)))
    return assemble_output(res.results)
